# revision 1
# baseline (speedup 1.0000x reference)
"""KAN forward kernel for 8 Trainium2 NeuronCores.

Strategy: data-parallel over N=32768 rows (4096 rows/core), weights
replicated. On-chip layout is transposed: features on partitions, rows on
the free dimension. Each KAN layer = one fused GEMM accumulation in PSUM:
  out[o, n] = sum_f bw[o,f]*silu(h[f,n]) + sum_{f,j} swc[o,f,j]*B_j(h[f,n])
B_j(h) = N(u-j), u = 2.5h+5.5, where N is the cardinal cubic B-spline
evaluated exactly as (r^3 - 4 t^3)/6 with r=relu(2-s), t=relu(r-1),
s=|u-(j+2)|: ScalarE does Abs and Relu (free affine), one 8-op custom
VectorE pass does the cubes. Sin/cos positional encoding uses exact
Cody-Waite range reduction (magic-number round) + the ScalarE Sin table.
GEMMs run in float32r (full PE rate, ~1e-3 elem precision).
"""
import numpy as np

import concourse.bacc as bacc
import concourse.mybir as mybir
import concourse.tile as tile
from concourse import bass_utils
from concourse.dve_spec import (
    Spec, Src0, Src1, C0, C1, C2, Zero, One, relu, sq, maxx, lower,
)
from concourse.dve_ops import DveOp, OPS
from concourse.dve_uop import DveOpSpec
from concourse.dve_spec import _has_src1 as has_src1

N_TOTAL = 32768
NCORES = 8
ROWS = N_TOTAL // NCORES          # 4096 rows per core
ENC = 16
DIMS = [(32, 256), (256, 256), (256, 256), (256, 1)]
CBRT4 = float(4.0 ** (1.0 / 3.0))
MAGIC = 12582912.0                # 1.5 * 2^23: fp32 round-to-nearest
TWO_PI = 2.0 * np.pi
# Cody-Waite split of 2*pi: c1 has 10 mantissa bits so k*c1 is exact
_c1_bits = np.float32(TWO_PI).view(np.uint32) & np.uint32(0xFFFFE000)
C1_2PI = float(_c1_bits.view(np.float32))
C2_2PI = float(np.float32(TWO_PI - C1_2PI))

f32 = mybir.dt.float32
f32r = mybir.dt.float32r
AF = mybir.ActivationFunctionType
CHUNK = 512                       # elementwise column chunk
RT = 512                          # psum row tile


def _make_op(name, spec):
    import concourse.dve_ops as dm
    for op in OPS:
        if op.name == name:
            return op
    shas = {}
    for ver in ("v3", "v4"):
        uops = lower(spec, ver=ver)
        shas[ver] = DveOpSpec(
            name=name, opcode=0, uops=uops, rd1_en=has_src1(spec)).sha(ver)
    op = DveOp(name, spec, subdim=False, uops_sha=shas)
    OPS.append(op)
    dm.CUSTOM_DVE_SPECS[name] = spec
    dm._SUB_OPCODE_FOR_NAME[name] = dm._CUSTOM_DVE_ROW_BASE + len(OPS) - 1
    assert dm._SUB_OPCODE_FOR_NAME[name] < 0x20
    return op


def _register_ops():
    # basis: in0 = r = relu(2-|u-c|); out = r^3 - (cbrt4*relu(r-1))^3 = 6*N
    _t = relu(Src0 - One) * C0
    bspl2 = _make_op("KAN_BSPL2", Spec(body=sq(Src0) * Src0 - sq(_t) * _t))
    # fused prep: r = relu(2 - |2.5*h + coff|)  (replaces ACT Abs+Relu)
    _m = Src0 * C0 + C1
    _sv = maxx(_m, Zero - _m)
    bspl1 = _make_op("KAN_BSPL1", Spec(body=relu(C2 - _sv)))
    # encoder pass1: k = round(x*qscale[p] + turns[p])  (magic rounding)
    _q = Src0 * C0 + C1
    enc1 = _make_op("KAN_ENC1", Spec(body=(_q + C2) - C2))
    # encoder pass2: t = (x*freq[p] - k*c1) - k*c2
    enc2 = _make_op(
        "KAN_ENC2", Spec(body=(Src0 * C0 - Src1 * C1) - Src1 * C2))
    return bspl2, bspl1, enc1, enc2


_CACHE = {}


def _build():
    if "nc" in _CACHE:
        return _CACHE["nc"], _CACHE["names"]
    BSPL2, BSPL1, ENC1, ENC2 = _register_ops()
    nc = bacc.Bacc("TRN2", target_bir_lowering=False, debug=False,
                   num_devices=NCORES)

    def reg_const(value, dtype=f32):
        if (dtype, value) in nc.const_aps.aps:
            return
        t = nc.alloc_sbuf_tensor(f"const-{dtype.name}-{value}", [128, 1], dtype)
        nc.gpsimd.memset(t.ap(), value)
        nc.const_aps.aps[(dtype, value)] = t.ap()

    for j in range(8):
        reg_const(3.5 - j)
    for v in (2.0, 2.5, -1.0):
        reg_const(v)

    # ---- DRAM I/O ----
    d_xrep = nc.dram_tensor("xrep", [32, ROWS], f32, kind="ExternalInput")
    d_encq = nc.dram_tensor("encq", [32, 1], f32, kind="ExternalInput")
    d_enct = nc.dram_tensor("enct", [32, 1], f32, kind="ExternalInput")
    d_encf = nc.dram_tensor("encf", [32, 1], f32, kind="ExternalInput")
    d_encb = nc.dram_tensor("encb", [32, 1], f32, kind="ExternalInput")
    d_l0b = [nc.dram_tensor(f"l0b{i}", [128, 1], f32, kind="ExternalInput")
             for i in range(2)]
    d_wb, d_ws = [], []
    for li, (fin, fout) in enumerate(DIMS):
        d_wb.append(nc.dram_tensor(f"wb{li}", [fin, fout], f32,
                                   kind="ExternalInput"))
        d_ws.append(nc.dram_tensor(f"ws{li}", [8 * fin, fout], f32,
                                   kind="ExternalInput"))
    d_out = nc.dram_tensor("out", [1, ROWS], f32, kind="ExternalOutput")

    with tile.TileContext(nc) as tc:
        with tc.tile_pool(name="wpool", bufs=1) as wp, \
             tc.tile_pool(name="hpool", bufs=1) as hp, \
             tc.tile_pool(name="plane", bufs=1) as plp, \
             tc.tile_pool(name="small", bufs=1) as sp, \
             tc.tile_pool(name="work", bufs=2) as wkp, \
             tc.tile_pool(name="psum", bufs=4, space="PSUM") as pp:

            # ---- load + round weights to f32r ----
            wb, ws = [], []
            for li, (fin, fout) in enumerate(DIMS):
                nkt = (fin + 127) // 128
                kb = []
                for kt in range(nkt):
                    p = min(128, fin - kt * 128)
                    tf = wp.tile([128, 256], f32, tag="wstage", name=f"wb{li}{kt}f")[:p, :fout]
                    nc.sync.dma_start(tf[:], d_wb[li].ap()[kt*128:kt*128+p, :])
                    tr = wp.tile([p, fout], f32r, tag=f"wb{li}{kt}r", name=f"wb{li}{kt}r")
                    nc.vector.tensor_copy(tr[:], tf[:])
                    kb.append(tr)
                wb.append(kb)
                if li == 0:
                    # two stacked quad tiles: rows [0:128] = j 0..3, [128:256] = j 4..7
                    kj = []
                    for i in range(2):
                        tf = wp.tile([128, 256], f32, tag="wstage", name=f"ws0q{i}f")[:, :fout]
                        nc.sync.dma_start(tf[:], d_ws[0].ap()[i*128:i*128+128, :])
                        tr = wp.tile([128, fout], f32r, name=f"ws0q{i}r")
                        nc.vector.tensor_copy(tr[:], tf[:])
                        kj.append(tr)
                    ws.append(kj)
                else:
                    kj = []
                    for j in range(8):
                        row = []
                        for kt in range(nkt):
                            p = min(128, fin - kt * 128)
                            off = j * fin + kt * 128
                            tf = wp.tile([128, 256], f32, tag="wstage", name=f"ws{li}{j}{kt}f")[:p, :fout]
                            nc.sync.dma_start(tf[:], d_ws[li].ap()[off:off+p, :])
                            tr = wp.tile([p, fout], f32r, name=f"ws{li}{j}{kt}r")
                            nc.vector.tensor_copy(tr[:], tf[:])
                            row.append(tr)
                        kj.append(row)
                    ws.append(kj)

            # ---- encoder consts ----
            t_q = sp.tile([32, 1], f32, name="t_q")
            t_tn = sp.tile([32, 1], f32, name="t_tn")
            t_f = sp.tile([32, 1], f32, name="t_f")
            t_b = sp.tile([32, 1], f32, name="t_b")
            nc.sync.dma_start(t_q[:], d_encq.ap())
            nc.sync.dma_start(t_tn[:], d_enct.ap())
            nc.sync.dma_start(t_f[:], d_encf.ap())
            nc.sync.dma_start(t_b[:], d_encb.ap())
            t_l0b = [sp.tile([128, 1], f32, name=f"l0b{i}") for i in range(2)]
            for i in range(2):
                nc.sync.dma_start(t_l0b[i][:], d_l0b[i].ap())

            h_cur = None  # list of (128, ROWS) tiles for layers 1..3

            for li, (fin, fout) in enumerate(DIMS):
                nkt = (fin + 127) // 128
                n_mt = (fout + 127) // 128
                if li == 0:
                    out_tiles = [hp.tile([128, ROWS], f32, tag=f"h{li%2}m{m}", name=f"h{li}m{m}")
                                 for m in range(n_mt)]
                elif li < 3:
                    out_tiles = [hp.tile([128, ROWS], f32, tag=f"h{li%2}m{m}", name=f"h{li}m{m}")
                                 for m in range(n_mt)]
                else:
                    out_tiles = [hp.tile([1, ROWS], f32, tag="hout", name="hout")]

                for ch in range(ROWS // CHUNK):
                    cs = ch * CHUNK
                    # --- elementwise planes for this chunk ---
                    if li == 0:
                        # encode this chunk: x -> h0 chunk, replicate to quads
                        t_x = wkp.tile([32, CHUNK], f32, tag="encx", name="encx")
                        nc.sync.dma_start(t_x[:], d_xrep.ap()[:, cs:cs+CHUNK])
                        t_k = wkp.tile([32, CHUNK], f32, tag="enck", name="enck")
                        nc.vector._custom_dve(ENC1, out=t_k[:], in0=t_x[:],
                                              s0=t_q[:], s1=t_tn[:], imm2=MAGIC)
                        t_red = wkp.tile([32, CHUNK], f32, tag="encr", name="encr")
                        nc.vector._custom_dve(ENC2, out=t_red[:], in0=t_x[:],
                                              in1=t_k[:], s0=t_f[:],
                                              s1=C1_2PI, imm2=C2_2PI)
                        h0c = wkp.tile([32, CHUNK], f32, tag="h0c", name="h0c")
                        nc.scalar.activation(h0c[:], t_red[:], AF.Sin, bias=t_b[:])
                        planes = []   # [(tile, psize, weight)] K-planes
                        silu = plp.tile([32, CHUNK], f32r, tag="silu0", name="silu0")
                        nc.scalar.activation(silu[:], h0c[:], AF.Silu)
                        planes.append((silu, 32, wb[0][0]))
                        for i in range(2):
                            rep = wkp.tile([128, CHUNK], f32, tag=f"rep{i}", name=f"rep{i}")
                            for q in range(4):
                                nc.sync.dma_start(rep[32*q:32*q+32, :], h0c[:])
                            s_t = wkp.tile([128, CHUNK], f32, tag="s0", name="s0")
                            nc.scalar.activation(
                                s_t[:], rep[:], AF.Abs,
                                bias=t_l0b[i][:], scale=2.5)
                            r_t = wkp.tile([128, CHUNK], f32, tag="r0", name="r0")
                            nc.scalar.activation(
                                r_t[:], s_t[:], AF.Relu, bias=2.0, scale=-1.0)
                            b_t = plp.tile([128, CHUNK], f32r, tag=f"bq{i}", name=f"bq{i}")
                            nc.vector._custom_dve(
                                BSPL2, out=b_t[:], in0=r_t[:], s0=CBRT4)
                            planes.append((b_t, 128, None))
                    else:
                        planes = []
                        for kt in range(nkt):
                            hsrc = h_cur[kt][:, cs:cs+CHUNK]
                            silu = plp.tile([128, CHUNK], f32r,
                                            tag=f"silu{kt}")
                            nc.scalar.activation(silu[:], hsrc, AF.Silu)
                            planes.append((silu, 128, wb[li][kt]))
                        for j in range(8):
                            for kt in range(nkt):
                                hsrc = h_cur[kt][:, cs:cs+CHUNK]
                                if j < 4:
                                    s_t = wkp.tile([128, CHUNK], f32,
                                                   tag="sa")
                                    nc.scalar.activation(
                                        s_t[:], hsrc, AF.Abs,
                                        bias=float(3.5 - j), scale=2.5)
                                    r_t = wkp.tile([128, CHUNK], f32,
                                                   tag="ra")
                                    nc.scalar.activation(
                                        r_t[:], s_t[:], AF.Relu,
                                        bias=2.0, scale=-1.0)
                                else:
                                    r_t = wkp.tile([128, CHUNK], f32,
                                                   tag="rd")
                                    nc.vector._custom_dve(
                                        BSPL1, out=r_t[:], in0=hsrc,
                                        s0=2.5, s1=float(3.5 - j), imm2=2.0)
                                b_t = plp.tile([128, CHUNK], f32r,
                                               tag=f"b{j}_{kt}")
                                nc.vector._custom_dve(
                                    BSPL2, out=b_t[:], in0=r_t[:], s0=CBRT4)
                                planes.append((b_t, 128, ws[li][j][kt]))

                    # --- GEMMs: accumulate all K-planes into psum ---
                    for sub in range(CHUNK // RT):
                        ss = sub * RT
                        for m in range(n_mt):
                            mp = min(128, fout - m * 128)
                            ps = pp.tile([mp, RT], f32, tag=f"ps{m}", name=f"ps{m}")
                            if li == 0:
                                mm = []
                                mm.append((planes[0][0][:, ss:ss+RT],
                                           wb[0][0][:, m*128:m*128+mp]))
                                for i in range(2):
                                    mm.append((planes[1+i][0][:, ss:ss+RT],
                                               ws[0][i][:, m*128:m*128+mp]))
                            else:
                                mm = [(pt[:, ss:ss+RT],
                                       wt[:, m*128:m*128+mp])
                                      for (pt, psz, wt) in planes]
                            nmm = len(mm)
                            for i, (rhs, lhsT) in enumerate(mm):
                                nc.tensor.matmul(
                                    ps[:], lhsT, rhs,
                                    start=(i == 0), stop=(i == nmm - 1))
                            dst = out_tiles[m][:, cs+ss:cs+ss+RT]
                            nc.vector.tensor_copy(dst, ps[:])
                h_cur = out_tiles

            nc.sync.dma_start(d_out.ap(), h_cur[0][:])

    nc.compile()
    _CACHE["nc"] = nc
    _CACHE["names"] = None
    return nc, None


def _host_inputs(x, freq, layer_params):
    """Build per-core input maps (host-side shard + weight transform)."""
    ins = {}
    qscale = np.zeros((32, 1), np.float32)
    fr = np.zeros((32, 1), np.float32)
    turns = np.zeros((32, 1), np.float32)
    sbias = np.zeros((32, 1), np.float32)
    fq = freq.astype(np.float32).reshape(-1)
    qscale[:16, 0] = fq / np.float32(TWO_PI)
    qscale[16:, 0] = fq / np.float32(TWO_PI)
    fr[:16, 0] = fq
    fr[16:, 0] = fq
    turns[16:, 0] = 0.25
    sbias[16:, 0] = np.pi / 2
    ins["encq"], ins["encf"] = qscale, fr
    ins["enct"], ins["encb"] = turns, sbias
    l0b0 = (3.5 - (np.arange(128) // 32)).astype(np.float32).reshape(128, 1)
    l0b1 = (3.5 - (np.arange(128) // 32 + 4)).astype(np.float32).reshape(128, 1)
    ins["l0b0"], ins["l0b1"] = l0b0, l0b1
    for li, (bw, sw, ss) in enumerate(layer_params):
        fout, fin = bw.shape
        ins[f"wb{li}"] = np.ascontiguousarray(bw.T.astype(np.float32))
        swc = (sw * ss[..., None]).astype(np.float32) / 6.0  # (O, F, 8)
        wsp = np.transpose(swc, (2, 1, 0)).reshape(8 * fin, fout)
        ins[f"ws{li}"] = np.ascontiguousarray(wsp)
    in_maps = []
    for c in range(NCORES):
        m = dict(ins)
        xc = x[c*ROWS:(c+1)*ROWS, 0].astype(np.float32)
        m["xrep"] = np.ascontiguousarray(
            np.broadcast_to(xc[None, :], (32, ROWS)))
        in_maps.append(m)
    return in_maps


def kernel(x, freq, bw0, sw0, ss0, bw1, sw1, ss1, bw2, sw2, ss2,
           bw3, sw3, ss3, **_):
    x = np.asarray(x, np.float32)
    layers = [(np.asarray(bw0), np.asarray(sw0), np.asarray(ss0)),
              (np.asarray(bw1), np.asarray(sw1), np.asarray(ss1)),
              (np.asarray(bw2), np.asarray(sw2), np.asarray(ss2)),
              (np.asarray(bw3), np.asarray(sw3), np.asarray(ss3))]
    nc, _names = _build()
    in_maps = _host_inputs(x, np.asarray(freq), layers)
    res = bass_utils.run_bass_kernel_spmd(
        nc, in_maps, core_ids=list(range(NCORES)))
    out = np.concatenate(
        [res.results[c]["out"].reshape(ROWS, 1) for c in range(NCORES)], 0)
    return out.astype(np.float32)

